# revision 71
# baseline (speedup 1.0000x reference)
"""CAM (channel attention) kernel for Trainium2, 8-core data-parallel over batch.

Per batch item (one per NeuronCore):
    energy   = Q @ K^T                     (C x C, contract over N)
    att      = softmax(max(energy) - energy) = softmax(-energy)   (shift-invariant)
    out      = gamma * (att @ V) + V

Pipeline layout (q,k,v: [C=512, N=4096] f32 in DRAM). The DMA engines are
the serial bottleneck resource (32 MB of HBM traffic/core), so the schedule
keeps them streaming continuously and keeps the PE warm (no p-state resets):

  front (DMA-paced, ~47us of q/k loads): per n-group of GJ=8 128-chunks,
    q,k stream in as [128,1024] f32 (q on sync/HWDGE, k on gpsimd/SWDGE),
    are cast to bf16 (DVE for q, ACT for k), then block-transposed on the
    PE (identity matmul, 4 blocks batched per PSUM tile) with PSUM->SBUF
    copies alternating DVE/ACT. mm1 for group g follows its transposes in
    the PE stream, so loads(g+1) overlap transpose+mm1(g) and the PE never
    idles. energy accumulates in 4 PSUM banks ([128c, 512d]) over all jj.
    The last group's mm1 runs c-tile-major so softmax(c0) overlaps mm1(c1+).
  softmax(-energy): DVE row-min, ACT exp(bias=rowmin, scale=-1) with fused
    row-sum, DVE reciprocal; gamma folds into the normalization scale so
    mm2 directly produces gamma*(att@V); att is bf16.
  att transposed via PE (4 blocks batched per PSUM tile) into attT[d].
  mm2 (PE-paced, v streamed): per 512-wide output chunk, v slabs
    [128,512] f32 load just-in-time (sync/gpsimd), cast to bf16 on ACT,
    4 accumulating matmuls per c-tile, epilogue out = psum + v (exact f32)
    on DVE, stores per [128, 4x512] chunk overlap the remaining matmuls.
"""

import numpy as np

B, C, H, W = 8, 512, 64, 64
N = H * W  # 4096
P = 128
CT = C // P  # 4 c-tiles
NJ = N // P  # 32 n-chunks
NO = N // 512  # 8 output column chunks

_nc_cache: dict = {}


def _body(nc, tc, cfg):
    from contextlib import ExitStack

    import concourse.mybir as mybir
    from concourse.bass import ts
    from concourse.masks import make_identity

    cfg = cfg or {}
    do = lambda phase: phase not in cfg.get("skip", ())

    def stage_bound():
        # explicit staggered-reset stage seam (only meaningful inside For_i)
        if cfg.get("stage_bounds") and getattr(tc, "_cur_loop_inst", None) is not None:
            tc.stage_boundary()

    dt = mybir.dt
    f32, bf16, f32r = dt.float32, dt.bfloat16, dt.float32r
    X = mybir.AxisListType.X

    if "ng" in cfg:
        ng = cfg["ng"]
        groups = [NJ // ng] * ng
    else:
        # front pipeline group sizes (in 128-col n-chunks); the small last
        # group shortens the serial mm1-tail after the final k/q load
        groups = cfg.get("groups", [4, 4, 4, 4, 4, 4, 4, 2, 2])
    assert sum(groups) == NJ
    NG = len(groups)
    GM = max(groups)  # tile allocation size

    qa_p = nc.kio["q"].ap().rearrange("(a p) w -> p a w", p=P)
    ka_p = nc.kio["k"].ap().rearrange("(a p) w -> p a w", p=P)
    va_p = nc.kio["v"].ap().rearrange("(a p) w -> p a w", p=P)
    ga = nc.kio["gamma"].ap()
    oa_p = nc.kio["out"].ap().rearrange("(a p) w -> p a w", p=P)

    with ExitStack() as ctx:
        ep = ctx.enter_context

        p_st32 = ep(tc.tile_pool(name="st32", bufs=cfg.get("st32_bufs", 2)))
        p_st16 = ep(tc.tile_pool(name="st16", bufs=cfg.get("st16_bufs", 2)))
        p_T = ep(tc.tile_pool(name="pT", bufs=1))
        p_att = ep(tc.tile_pool(name="att", bufs=CT))
        p_attT = ep(tc.tile_pool(name="attT", bufs=CT))
        p_small = ep(tc.tile_pool(name="small", bufs=2))
        p_misc = ep(tc.tile_pool(name="misc", bufs=1))
        p_vf = ep(tc.tile_pool(name="vf", bufs=cfg.get("vf_bufs", 3)))
        p_vb = ep(tc.tile_pool(name="vb", bufs=cfg.get("vb_bufs", 2)))
        p_es = ep(tc.tile_pool(name="es", bufs=cfg.get("es_bufs", 3)))

        # gamma broadcast across partitions: [1,1] DRAM -> [128,1] SBUF
        g128 = p_misc.tile([P, 1], f32)
        nc.sync.dma_start(g128[:], ga.broadcast_to([P, 1]))

        ident = p_misc.tile([P, P], bf16)
        make_identity(nc, ident[:])

        # packed transposed tensors, one tile per n-group:
        # qT[g][:, c, jj, :] is the [128n, 128c] lhsT for mm1 (g, jj, c)
        # kT[g][:, jj] is the [128n, 512d] rhs for mm1 (g, jj)
        qT = [
            p_T.tile([P, CT, GM, P], bf16, tag="qT", bufs=2, name=f"qT{g}")
            for g in range(NG)
        ]
        kT = [
            p_T.tile([P, GM, CT, P], bf16, tag="kT", bufs=2, name=f"kT{g}")
            for g in range(NG)
        ]

        att = []
        with tc.tile_pool(name="energy", bufs=CT, space="PSUM") as p_energy, \
             tc.tile_pool(name="ptp", bufs=cfg.get("ptp_bufs", 4), space="PSUM") as p_ptp:
            e_ps = [
                p_energy.tile([P, 512], f32, tag="e", name=f"e{c}")
                for c in range(CT)
            ]

            copy2 = [nc.vector.tensor_copy, nc.scalar.copy]
            alt = 0
            j0 = 0
            for g, gj in enumerate(groups):
                gw = gj * P
                # one batched [128, CT, gw] load per tensor per group (sync
                # FIFO, k before q); casts per c-tile (q: DVE, k: ACT)
                kn32 = p_st32.tile([P, CT, GM * P], f32, tag="kn32", name=f"kn32_{g}")
                nc.sync.dma_start(
                    kn32[:, :, 0:gw], ka_p[:, :, j0 * P : j0 * P + gw]
                )
                k16 = p_st16.tile([P, CT, GM * P], bf16, tag="kn", name=f"kn{g}")
                qn32 = p_st32.tile([P, CT, GM * P], f32, tag="qn32", name=f"qn32_{g}")
                nc.sync.dma_start(
                    qn32[:, :, 0:gw], qa_p[:, :, j0 * P : j0 * P + gw]
                )
                q16 = p_st16.tile([P, CT, GM * P], bf16, tag="qn", name=f"qn{g}")
                # casts split across ACT and DVE so the last group's cast
                # latency halves (k: ACT gets c0/c1; q: DVE gets c0/c1)
                for c in range(CT):
                    if c < 2:
                        nc.scalar.copy(k16[:, c, 0:gw], kn32[:, c, 0:gw])
                    else:
                        nc.vector.tensor_copy(k16[:, c, 0:gw], kn32[:, c, 0:gw])
                for c in range(CT):
                    if c < 2:
                        nc.vector.tensor_copy(q16[:, c, 0:gw], qn32[:, c, 0:gw])
                    else:
                        nc.scalar.copy(q16[:, c, 0:gw], qn32[:, c, 0:gw])
                if not do("tpose"):
                    j0 += gj
                    continue
                # PE block-transposes, up to 4 blocks batched per PSUM tile.
                # k first (mm1 (g, jj, c) needs kT[g][:, jj] for all c).
                for jj in range(gj):
                    ptp = p_ptp.tile([P, CT, P], bf16, tag="ptp")
                    for c in range(CT):
                        nc.tensor.transpose(
                            ptp[:, c], k16[:, c, ts(jj, P)], ident[:]
                        )
                    copy2[alt % 2](kT[g][:, jj], ptp[:])
                    alt += 1
                for c in range(CT):
                    for h0 in range(0, gj, 4):
                        nb = min(4, gj - h0)
                        ptp = p_ptp.tile([P, 4, P], bf16, tag="ptp")
                        for i in range(nb):
                            nc.tensor.transpose(
                                ptp[:, i], q16[:, c, ts(h0 + i, P)], ident[:]
                            )
                        copy2[alt % 2](
                            qT[g][:, c, h0 : h0 + nb, :], ptp[:, 0:nb]
                        )
                        alt += 1
                if not do("mm1"):
                    j0 += gj
                    continue
                # mm1: energy[c] += qT[:,c,jj,:].T @ kT[:,jj]
                # last group c-major so softmax(c) can start as e_ps[c] stops
                if g == NG - 1:
                    for c in range(CT):
                        for jj in range(gj):
                            nc.tensor.matmul(
                                e_ps[c][:],
                                qT[g][:, c, jj, :],
                                kT[g][:, jj],
                                start=False,
                                stop=(jj == gj - 1),
                            )
                else:
                    for jj in range(gj):
                        for c in range(CT):
                            nc.tensor.matmul(
                                e_ps[c][:],
                                qT[g][:, c, jj, :],
                                kT[g][:, jj],
                                start=(g == 0 and jj == 0),
                                stop=False,
                            )
                if g == 3:
                    stage_bound()  # seam 1: mid-front
                j0 += gj

            # v chunk loads (4 x 2MB) queue on sync right behind the last
            # q/k load: they fill the DMA pipe during the mm1 tail + softmax
            VCW = cfg.get("vcw", 1024)  # columns per v chunk
            vch = []
            if do("mm2"):
                for vi in range(N // VCW):
                    vc = p_vf.tile(
                        [P, CT, VCW], f32, tag="vch", name=f"vch{vi}"
                    )
                    nc.sync.dma_start(
                        vc[:], va_p[:, :, ts(vi, VCW)]
                    )
                    vch.append(vc)
                stage_bound()  # seam 2: end of loads, pre-softmax

            if not (do("tpose") and do("mm1")):
                return

            if cfg.get("dump_energy"):
                oa = nc.kio["out"].ap().rearrange("(a p) w -> a p w", p=P)
                for c in range(CT):
                    ed = p_es.tile([P, 512], f32, tag="ed", name=f"ed{c}")
                    nc.vector.tensor_copy(ed[:], e_ps[c][:])
                    nc.sync.dma_start(oa[c][:, 0:512], ed[:])
                return

            # keep the PE (HAM activity window) warm through the ~5us
            # softmax gap: a few discarded matmuls on resident data so mm2
            # doesn't restart at the throttled p-state
            if cfg.get("warm_mms", 0):
                junk = p_ptp.tile([P, 512], f32, tag="warm", bufs=1)
                for w in range(cfg.get("warm_mms", 0)):
                    nc.tensor.matmul(
                        junk[:],
                        ident[:],
                        kT[NG - 1][:, 0].rearrange("p a b -> p (a b)"),
                        start=True,
                        stop=True,
                    )

            # softmax(-energy) rows, gamma folded into the normalization
            for c in range(CT):
                rowmin = p_small.tile([P, 1], f32)
                nc.vector.tensor_reduce(
                    rowmin[:], e_ps[c][:], axis=X, op=mybir.AluOpType.min
                )
                pexp = p_att.tile([P, 512], bf16, tag="att", name=f"att{c}")
                rowsum = p_small.tile([P, 1], f32)
                nc.scalar.activation(
                    pexp[:],
                    e_ps[c][:],
                    mybir.ActivationFunctionType.Exp,
                    bias=rowmin[:, 0:1],
                    scale=-1.0,
                    accum_out=rowsum[:, 0:1],
                )
                recip = p_small.tile([P, 1], f32)
                nc.vector.reciprocal(recip[:], rowsum[:])
                srow = p_small.tile([P, 1], f32)
                nc.vector.tensor_scalar_mul(srow[:], recip[:], g128[:, 0:1])
                nc.vector.tensor_scalar_mul(pexp[:], pexp[:], srow[:, 0:1])
                att.append(pexp)

        if cfg.get("dump_att"):
            ob = nc.kio["out"].ap().bitcast(bf16).rearrange("(a p) w -> a p w", p=P)
            for c in range(CT):
                nc.sync.dma_start(ob[c][:, 0:512], att[c][:])
            return

        if not do("mm2"):
            return

        # transpose att (bf16) via PE into attT[d][:, c-block]
        attT = []
        with tc.tile_pool(name="pst", bufs=2, space="PSUM") as p_pst:
            for d in range(CT):
                pst = p_pst.tile([P, CT, P], bf16, tag="pst")
                for c in range(CT):
                    nc.tensor.transpose(
                        pst[:, c], att[c][:, ts(d, P)], ident[:]
                    )
                at = p_attT.tile([P, C], bf16, tag="attT", name=f"attT{d}")
                if d % 2:
                    nc.vector.tensor_copy(at[:], pst[:])
                else:
                    nc.scalar.copy(at[:], pst[:])
                attT.append(at)

        with tc.tile_pool(name="ps2", bufs=cfg.get("ps2_bufs", 6), space="PSUM") as p_ps2:
            # mm2 (bf16): psum = gamma*(att @ V); epilogue adds v (exact f32)
            # on DVE. v was preloaded in 2MB chunks; per-(no,d) bf16 casts on
            # ACT; one [128, CT, 512] (1MB) store per chunk on the gpsimd
            # queue (its own queue: a store waiting on es4 blocks nothing).
            for no in range(NO):
                if no == 4:
                    stage_bound()  # seam 3: mid-mm2
                vc = vch[no * 512 // VCW]
                off = (no * 512) % VCW
                vbs = []
                for d in range(CT):
                    vb = p_vb.tile([P, 512], bf16, tag=f"vb{d}", name=f"vb{d}_{no}")
                    if cfg.get("vb_split") and d % 2:
                        nc.gpsimd.tensor_copy(vb[:], vc[:, d, off : off + 512])
                    else:
                        nc.scalar.copy(vb[:], vc[:, d, off : off + 512])
                    vbs.append(vb)
                if cfg.get("es_pair"):
                    # pair two output chunks -> one 2MB store (4KB segments)
                    if no % 2 == 0:
                        es4 = p_es.tile(
                            [P, CT, 1024], f32, tag="es8", name=f"es8_{no // 2}"
                        )
                    off2 = (no % 2) * 512
                else:
                    es4 = p_es.tile(
                        [P, CT, 512], f32, tag="es4", name=f"es4_{no}"
                    )
                    off2 = 0
                for c in range(CT):
                    ps2 = p_ps2.tile([P, 512], f32, tag="ps2")
                    for d in range(CT):
                        nc.tensor.matmul(
                            ps2[:],
                            attT[d][:, ts(c, P)],
                            vbs[d][:],
                            start=(d == 0),
                            stop=(d == CT - 1),
                        )
                    nc.vector.tensor_add(
                        es4[:, c, off2 : off2 + 512],
                        ps2[:],
                        vc[:, c, off : off + 512],
                    )
                # single sync HWDGE queue: the store blocks nothing behind it
                if cfg.get("es_pair"):
                    if no % 2 == 1:
                        nc.sync.dma_start(
                            oa_p[:, :, (no - 1) * 512 : (no + 1) * 512], es4[:]
                        )
                else:
                    nc.sync.dma_start(oa_p[:, :, ts(no, 512)], es4[:])


def build(repeat=1, cfg=None, loop_n=None):
    import concourse.mybir as mybir
    import concourse.tile as tile
    from concourse import bacc

    dt = mybir.dt
    nc = bacc.Bacc("TRN2", target_bir_lowering=False, debug=False)
    nc.kio = {}
    for name in ("q", "k", "v"):
        nc.kio[name] = nc.dram_tensor(
            name, [C, N], dt.float32, kind="ExternalInput"
        )
    nc.kio["gamma"] = nc.dram_tensor(
        "gamma", [1, 1], dt.float32, kind="ExternalInput"
    )
    nc.kio["out"] = nc.dram_tensor(
        "out", [C, N], dt.float32, kind="ExternalOutput"
    )
    with tile.TileContext(nc) as tc:
        if loop_n is not None:
            # staggered_reset: per-stage semaphore resets instead of one
            # all-engine back-edge barrier, so iteration i+1's loads overlap
            # iteration i's store tail
            with tc.For_i(0, loop_n, 1, staggered_reset=(cfg or {}).get("sr", True)):
                _body(nc, tc, cfg)
        else:
            for _ in range(repeat):
                _body(nc, tc, cfg)
    nc.compile()
    return nc


def _get_nc():
    if "nc" not in _nc_cache:
        _nc_cache["nc"] = build(repeat=1)
    return _nc_cache["nc"]


def make_in_maps(q, k, v, gamma):
    q = np.ascontiguousarray(np.asarray(q, dtype=np.float32).reshape(B, C, N))
    k = np.ascontiguousarray(np.asarray(k, dtype=np.float32).reshape(B, C, N))
    v = np.ascontiguousarray(np.asarray(v, dtype=np.float32).reshape(B, C, N))
    g = np.asarray(gamma, dtype=np.float32).reshape(1, 1)
    return [
        {"q": q[i], "k": k[i], "v": v[i], "gamma": g} for i in range(B)
    ]


def kernel(q, k, v, gamma):
    from concourse import bass_utils

    nc = _get_nc()
    in_maps = make_in_maps(q, k, v, gamma)
    res = bass_utils.run_bass_kernel_spmd(nc, in_maps, core_ids=list(range(B)))
    out = np.stack([res.results[i]["out"] for i in range(B)])
    return out.reshape(B, C, H, W).astype(np.float32, copy=False)


# revision 72
# speedup vs baseline: 1.0627x; 1.0627x over previous
"""CAM (channel attention) kernel for Trainium2, 8-core data-parallel over batch.

Per batch item (one per NeuronCore):
    energy   = Q @ K^T                     (C x C, contract over N)
    att      = softmax(max(energy) - energy) = softmax(-energy)   (shift-invariant)
    out      = gamma * (att @ V) + V

Pipeline layout (q,k,v: [C=512, N=4096] f32 in DRAM). The DMA engines are
the serial bottleneck resource (32 MB of HBM traffic/core), so the schedule
keeps them streaming continuously and keeps the PE warm (no p-state resets):

  front (DMA-paced, ~47us of q/k loads): per n-group of GJ=8 128-chunks,
    q,k stream in as [128,1024] f32 (q on sync/HWDGE, k on gpsimd/SWDGE),
    are cast to bf16 (DVE for q, ACT for k), then block-transposed on the
    PE (identity matmul, 4 blocks batched per PSUM tile) with PSUM->SBUF
    copies alternating DVE/ACT. mm1 for group g follows its transposes in
    the PE stream, so loads(g+1) overlap transpose+mm1(g) and the PE never
    idles. energy accumulates in 4 PSUM banks ([128c, 512d]) over all jj.
    The last group's mm1 runs c-tile-major so softmax(c0) overlaps mm1(c1+).
  softmax(-energy): DVE row-min, ACT exp(bias=rowmin, scale=-1) with fused
    row-sum, DVE reciprocal; gamma folds into the normalization scale so
    mm2 directly produces gamma*(att@V); att is bf16.
  att transposed via PE (4 blocks batched per PSUM tile) into attT[d].
  mm2 (PE-paced, v streamed): per 512-wide output chunk, v slabs
    [128,512] f32 load just-in-time (sync/gpsimd), cast to bf16 on ACT,
    4 accumulating matmuls per c-tile, epilogue out = psum + v (exact f32)
    on DVE, stores per [128, 4x512] chunk overlap the remaining matmuls.
"""

import numpy as np

B, C, H, W = 8, 512, 64, 64
N = H * W  # 4096
P = 128
CT = C // P  # 4 c-tiles
NJ = N // P  # 32 n-chunks
NO = N // 512  # 8 output column chunks

_nc_cache: dict = {}


def _body(nc, tc, cfg):
    from contextlib import ExitStack

    import concourse.mybir as mybir
    from concourse.bass import ts
    from concourse.masks import make_identity

    cfg = cfg or {}
    do = lambda phase: phase not in cfg.get("skip", ())

    def stage_bound():
        # explicit staggered-reset stage seam (only meaningful inside For_i)
        if cfg.get("stage_bounds") and getattr(tc, "_cur_loop_inst", None) is not None:
            tc.stage_boundary()

    dt = mybir.dt
    f32, bf16, f32r = dt.float32, dt.bfloat16, dt.float32r
    X = mybir.AxisListType.X

    if "ng" in cfg:
        ng = cfg["ng"]
        groups = [NJ // ng] * ng
    else:
        # front pipeline group sizes (in 128-col n-chunks); the small last
        # group shortens the serial mm1-tail after the final k/q load
        groups = cfg.get("groups", [4, 4, 4, 4, 4, 4, 4, 2, 2])
    assert sum(groups) == NJ
    NG = len(groups)
    GM = max(groups)  # tile allocation size

    qa_p = nc.kio["q"].ap().rearrange("(a p) w -> p a w", p=P)
    ka_p = nc.kio["k"].ap().rearrange("(a p) w -> p a w", p=P)
    va_p = nc.kio["v"].ap().rearrange("(a p) w -> p a w", p=P)
    ga = nc.kio["gamma"].ap()
    oa_p = nc.kio["out"].ap().rearrange("(a p) w -> p a w", p=P)

    with ExitStack() as ctx:
        ep = ctx.enter_context

        p_st32 = ep(tc.tile_pool(name="st32", bufs=cfg.get("st32_bufs", 2)))
        p_st16 = ep(tc.tile_pool(name="st16", bufs=cfg.get("st16_bufs", 2)))
        p_T = ep(tc.tile_pool(name="pT", bufs=1))
        p_att = ep(tc.tile_pool(name="att", bufs=CT))
        p_attT = ep(tc.tile_pool(name="attT", bufs=CT))
        p_small = ep(tc.tile_pool(name="small", bufs=2))
        p_misc = ep(tc.tile_pool(name="misc", bufs=1))
        p_vf = ep(tc.tile_pool(name="vf", bufs=cfg.get("vf_bufs", 3)))
        p_vb = ep(tc.tile_pool(name="vb", bufs=cfg.get("vb_bufs", 3)))
        p_es = ep(tc.tile_pool(name="es", bufs=cfg.get("es_bufs", 3)))

        # gamma broadcast across partitions: [1,1] DRAM -> [128,1] SBUF
        g128 = p_misc.tile([P, 1], f32)
        nc.sync.dma_start(g128[:], ga.broadcast_to([P, 1]))

        ident = p_misc.tile([P, P], bf16)
        make_identity(nc, ident[:])

        # packed transposed tensors, one tile per n-group:
        # qT[g][:, c, jj, :] is the [128n, 128c] lhsT for mm1 (g, jj, c)
        # kT[g][:, jj] is the [128n, 512d] rhs for mm1 (g, jj)
        qT = [
            p_T.tile([P, CT, GM, P], bf16, tag="qT", bufs=2, name=f"qT{g}")
            for g in range(NG)
        ]
        kT = [
            p_T.tile([P, GM, CT, P], bf16, tag="kT", bufs=2, name=f"kT{g}")
            for g in range(NG)
        ]

        att = []
        with tc.tile_pool(name="energy", bufs=CT, space="PSUM") as p_energy, \
             tc.tile_pool(name="ptp", bufs=cfg.get("ptp_bufs", 4), space="PSUM") as p_ptp:
            e_ps = [
                p_energy.tile([P, 512], f32, tag="e", name=f"e{c}")
                for c in range(CT)
            ]

            copy2 = [nc.vector.tensor_copy, nc.scalar.copy]
            alt = 0
            j0 = 0
            for g, gj in enumerate(groups):
                gw = gj * P
                # one batched [128, CT, gw] load per tensor per group (sync
                # FIFO, k before q); casts per c-tile (q: DVE, k: ACT)
                kn32 = p_st32.tile([P, CT, GM * P], f32, tag="kn32", name=f"kn32_{g}")
                nc.sync.dma_start(
                    kn32[:, :, 0:gw], ka_p[:, :, j0 * P : j0 * P + gw]
                )
                k16 = p_st16.tile([P, CT, GM * P], bf16, tag="kn", name=f"kn{g}")
                qn32 = p_st32.tile([P, CT, GM * P], f32, tag="qn32", name=f"qn32_{g}")
                nc.sync.dma_start(
                    qn32[:, :, 0:gw], qa_p[:, :, j0 * P : j0 * P + gw]
                )
                q16 = p_st16.tile([P, CT, GM * P], bf16, tag="qn", name=f"qn{g}")
                # casts split across ACT and DVE so the last group's cast
                # latency halves (k: ACT gets c0/c1; q: DVE gets c0/c1)
                for c in range(CT):
                    if c < 2:
                        nc.scalar.copy(k16[:, c, 0:gw], kn32[:, c, 0:gw])
                    else:
                        nc.vector.tensor_copy(k16[:, c, 0:gw], kn32[:, c, 0:gw])
                for c in range(CT):
                    if c < 2:
                        nc.vector.tensor_copy(q16[:, c, 0:gw], qn32[:, c, 0:gw])
                    else:
                        nc.scalar.copy(q16[:, c, 0:gw], qn32[:, c, 0:gw])
                if not do("tpose"):
                    j0 += gj
                    continue
                # PE block-transposes, up to 4 blocks batched per PSUM tile.
                # k first (mm1 (g, jj, c) needs kT[g][:, jj] for all c).
                for jj in range(gj):
                    ptp = p_ptp.tile([P, CT, P], bf16, tag="ptp")
                    for c in range(CT):
                        nc.tensor.transpose(
                            ptp[:, c], k16[:, c, ts(jj, P)], ident[:]
                        )
                    copy2[alt % 2](kT[g][:, jj], ptp[:])
                    alt += 1
                for c in range(CT):
                    for h0 in range(0, gj, 4):
                        nb = min(4, gj - h0)
                        ptp = p_ptp.tile([P, 4, P], bf16, tag="ptp")
                        for i in range(nb):
                            nc.tensor.transpose(
                                ptp[:, i], q16[:, c, ts(h0 + i, P)], ident[:]
                            )
                        copy2[alt % 2](
                            qT[g][:, c, h0 : h0 + nb, :], ptp[:, 0:nb]
                        )
                        alt += 1
                if not do("mm1"):
                    j0 += gj
                    continue
                # mm1: energy[c] += qT[:,c,jj,:].T @ kT[:,jj]
                # last group c-major so softmax(c) can start as e_ps[c] stops
                if g == NG - 1:
                    for c in range(CT):
                        for jj in range(gj):
                            nc.tensor.matmul(
                                e_ps[c][:],
                                qT[g][:, c, jj, :],
                                kT[g][:, jj],
                                start=False,
                                stop=(jj == gj - 1),
                            )
                else:
                    for jj in range(gj):
                        for c in range(CT):
                            nc.tensor.matmul(
                                e_ps[c][:],
                                qT[g][:, c, jj, :],
                                kT[g][:, jj],
                                start=(g == 0 and jj == 0),
                                stop=False,
                            )
                if g == 3:
                    stage_bound()  # seam 1: mid-front
                j0 += gj

            # v chunk loads (4 x 2MB) queue on sync right behind the last
            # q/k load: they fill the DMA pipe during the mm1 tail + softmax
            VCW = cfg.get("vcw", 1024)  # columns per v chunk
            vch = []
            if do("mm2"):
                for vi in range(N // VCW):
                    vc = p_vf.tile(
                        [P, CT, VCW], f32, tag="vch", name=f"vch{vi}"
                    )
                    nc.sync.dma_start(
                        vc[:], va_p[:, :, ts(vi, VCW)]
                    )
                    vch.append(vc)
                stage_bound()  # seam 2: end of loads, pre-softmax

            if not (do("tpose") and do("mm1")):
                return

            if cfg.get("dump_energy"):
                oa = nc.kio["out"].ap().rearrange("(a p) w -> a p w", p=P)
                for c in range(CT):
                    ed = p_es.tile([P, 512], f32, tag="ed", name=f"ed{c}")
                    nc.vector.tensor_copy(ed[:], e_ps[c][:])
                    nc.sync.dma_start(oa[c][:, 0:512], ed[:])
                return

            # keep the PE (HAM activity window) warm through the ~5us
            # softmax gap: a few discarded matmuls on resident data so mm2
            # doesn't restart at the throttled p-state
            if cfg.get("warm_mms", 0):
                junk = p_ptp.tile([P, 512], f32, tag="warm", bufs=1)
                for w in range(cfg.get("warm_mms", 0)):
                    nc.tensor.matmul(
                        junk[:],
                        ident[:],
                        kT[NG - 1][:, 0].rearrange("p a b -> p (a b)"),
                        start=True,
                        stop=True,
                    )

            # softmax(-energy) rows, gamma folded into the normalization
            for c in range(CT):
                rowmin = p_small.tile([P, 1], f32)
                nc.vector.tensor_reduce(
                    rowmin[:], e_ps[c][:], axis=X, op=mybir.AluOpType.min
                )
                pexp = p_att.tile([P, 512], bf16, tag="att", name=f"att{c}")
                rowsum = p_small.tile([P, 1], f32)
                nc.scalar.activation(
                    pexp[:],
                    e_ps[c][:],
                    mybir.ActivationFunctionType.Exp,
                    bias=rowmin[:, 0:1],
                    scale=-1.0,
                    accum_out=rowsum[:, 0:1],
                )
                recip = p_small.tile([P, 1], f32)
                nc.vector.reciprocal(recip[:], rowsum[:])
                srow = p_small.tile([P, 1], f32)
                nc.vector.tensor_scalar_mul(srow[:], recip[:], g128[:, 0:1])
                nc.vector.tensor_scalar_mul(pexp[:], pexp[:], srow[:, 0:1])
                att.append(pexp)

        if cfg.get("dump_att"):
            ob = nc.kio["out"].ap().bitcast(bf16).rearrange("(a p) w -> a p w", p=P)
            for c in range(CT):
                nc.sync.dma_start(ob[c][:, 0:512], att[c][:])
            return

        if not do("mm2"):
            return

        # transpose att (bf16) via PE into attT[d][:, c-block]
        attT = []
        with tc.tile_pool(name="pst", bufs=2, space="PSUM") as p_pst:
            for d in range(CT):
                pst = p_pst.tile([P, CT, P], bf16, tag="pst")
                for c in range(CT):
                    nc.tensor.transpose(
                        pst[:, c], att[c][:, ts(d, P)], ident[:]
                    )
                at = p_attT.tile([P, C], bf16, tag="attT", name=f"attT{d}")
                if d % 2:
                    nc.vector.tensor_copy(at[:], pst[:])
                else:
                    nc.scalar.copy(at[:], pst[:])
                attT.append(at)

        with tc.tile_pool(name="ps2", bufs=cfg.get("ps2_bufs", 8), space="PSUM") as p_ps2:
            # mm2 (bf16): psum = gamma*(att @ V); epilogue adds v (exact f32)
            # on DVE. v was preloaded in 2MB chunks; per-(no,d) bf16 casts on
            # ACT; one [128, CT, 512] (1MB) store per chunk on the gpsimd
            # queue (its own queue: a store waiting on es4 blocks nothing).
            for no in range(NO):
                if no == 4:
                    stage_bound()  # seam 3: mid-mm2
                vc = vch[no * 512 // VCW]
                off = (no * 512) % VCW
                vbs = []
                for d in range(CT):
                    vb = p_vb.tile([P, 512], bf16, tag=f"vb{d}", name=f"vb{d}_{no}")
                    if cfg.get("vb_split") and d % 2:
                        nc.gpsimd.tensor_copy(vb[:], vc[:, d, off : off + 512])
                    else:
                        nc.scalar.copy(vb[:], vc[:, d, off : off + 512])
                    vbs.append(vb)
                if cfg.get("es_pair"):
                    # pair two output chunks -> one 2MB store (4KB segments)
                    if no % 2 == 0:
                        es4 = p_es.tile(
                            [P, CT, 1024], f32, tag="es8", name=f"es8_{no // 2}"
                        )
                    off2 = (no % 2) * 512
                else:
                    es4 = p_es.tile(
                        [P, CT, 512], f32, tag="es4", name=f"es4_{no}"
                    )
                    off2 = 0
                for c in range(CT):
                    ps2 = p_ps2.tile([P, 512], f32, tag="ps2")
                    for d in range(CT):
                        nc.tensor.matmul(
                            ps2[:],
                            attT[d][:, ts(c, P)],
                            vbs[d][:],
                            start=(d == 0),
                            stop=(d == CT - 1),
                        )
                    nc.vector.tensor_add(
                        es4[:, c, off2 : off2 + 512],
                        ps2[:],
                        vc[:, c, off : off + 512],
                    )
                # single sync HWDGE queue: the store blocks nothing behind it
                if cfg.get("es_pair"):
                    if no % 2 == 1:
                        nc.sync.dma_start(
                            oa_p[:, :, (no - 1) * 512 : (no + 1) * 512], es4[:]
                        )
                else:
                    nc.sync.dma_start(oa_p[:, :, ts(no, 512)], es4[:])


def build(repeat=1, cfg=None, loop_n=None):
    import concourse.mybir as mybir
    import concourse.tile as tile
    from concourse import bacc

    dt = mybir.dt
    nc = bacc.Bacc("TRN2", target_bir_lowering=False, debug=False)
    nc.kio = {}
    for name in ("q", "k", "v"):
        nc.kio[name] = nc.dram_tensor(
            name, [C, N], dt.float32, kind="ExternalInput"
        )
    nc.kio["gamma"] = nc.dram_tensor(
        "gamma", [1, 1], dt.float32, kind="ExternalInput"
    )
    nc.kio["out"] = nc.dram_tensor(
        "out", [C, N], dt.float32, kind="ExternalOutput"
    )
    with tile.TileContext(nc) as tc:
        if loop_n is not None:
            # staggered_reset: per-stage semaphore resets instead of one
            # all-engine back-edge barrier, so iteration i+1's loads overlap
            # iteration i's store tail
            with tc.For_i(0, loop_n, 1, staggered_reset=(cfg or {}).get("sr", True)):
                _body(nc, tc, cfg)
        else:
            for _ in range(repeat):
                _body(nc, tc, cfg)
    nc.compile()
    return nc


def _get_nc():
    if "nc" not in _nc_cache:
        _nc_cache["nc"] = build(repeat=1)
    return _nc_cache["nc"]


def make_in_maps(q, k, v, gamma):
    q = np.ascontiguousarray(np.asarray(q, dtype=np.float32).reshape(B, C, N))
    k = np.ascontiguousarray(np.asarray(k, dtype=np.float32).reshape(B, C, N))
    v = np.ascontiguousarray(np.asarray(v, dtype=np.float32).reshape(B, C, N))
    g = np.asarray(gamma, dtype=np.float32).reshape(1, 1)
    return [
        {"q": q[i], "k": k[i], "v": v[i], "gamma": g} for i in range(B)
    ]


def kernel(q, k, v, gamma):
    from concourse import bass_utils

    nc = _get_nc()
    in_maps = make_in_maps(q, k, v, gamma)
    res = bass_utils.run_bass_kernel_spmd(nc, in_maps, core_ids=list(range(B)))
    out = np.stack([res.results[i]["out"] for i in range(B)])
    return out.reshape(B, C, H, W).astype(np.float32, copy=False)
